# revision 45
# baseline (speedup 1.0000x reference)
"""Trainium2 Bass kernel for AdaptiveReLU segment-reduce.

Reference computation (per segment s over instance rows x[i] with batch_idx[i]==s):
    mn = min, mx = max, sums = sum, n = count
    bias = t*mx + (1-t)*mn            (t clamped to [0,1], per feature)
    relu_sum = sum(relu(x - bias))
    out[s,f] = W0*n + W1*mn + W2*mx + W3*relu_sum + W4*sums

Strategy: host-side sort + count-sorted packing so every segment lives on one
core with a few % padding, then a fully local (collective-free) SPMD kernel on
8 NeuronCores.

Work split across engines:
  - DVE (feature layout, [128=(parity,feat), (j, block, g)] bf16): pairwise
    min/max trees, in-place clamp max(x, bias) (broadcast runs at 2x), and
    the relu-sum tree.  The clamp+relu phase of superblock k is deferred
    until after superblock k+1's trees so the PE path below can finish
    reading x tiles before they are clamped in place.
  - PE: raw segment sums via transpose-accumulate matmuls: for each
    position, sum_j transpose(x[:, j, :]) with an identity moving operand
    accumulates [seg-group, (parity, feat)] sums in PSUM, fp32-exact.
    Pad rows (which replicate row 0) are corrected via the host apl plane.
  - ACT: PSUM drains + secondary DMA queue.
  - Host: pack/unpack and the final out += W4*raw_sums add during unpack.
"""

import os
import numpy as np
import ml_dtypes

F = 64            # feature dim
G = 128           # segment-groups per position (2 parities x 64 features)
SPB = 2 * G       # segments per position per core
NCORES = 8
MAX_LM = 80       # SBUF cap: L * m <= MAX_LM  (tile = L*m*128 cols bf16)

BF16 = ml_dtypes.bfloat16


def _nfolds(L):
    n = 0
    while L > 1:
        if L % 2:
            n += 1
        L //= 2
    return n


def _partition(Ls):
    """DP partition of block positions into superblocks.

    Returns list of (start, m, Lpad).  Cost model (ns):
      padding: 270 per extra L-unit per position
      folds:   3 trees * (m*64 + 220) per odd level
      fixed:   5000 per superblock
    """
    NB = len(Ls)
    INF = float("inf")
    best = [INF] * (NB + 1)
    choice = [None] * (NB + 1)
    best[NB] = 0.0
    for i in range(NB - 1, -1, -1):
        for j in range(i + 1, NB + 1):
            m = j - i
            Lmax = int(Ls[i])
            if Lmax * m > MAX_LM:
                break
            c_best = INF
            lp_best = Lmax
            for Lp in range(Lmax, min(Lmax + 13, MAX_LM // m + 1)):
                pad = sum(Lp - int(Ls[k]) for k in range(i, j))
                c = pad * 270.0 + _nfolds(Lp) * 3 * (m * 64 + 220) + 5000.0
                if c < c_best:
                    c_best, lp_best = c, Lp
            if c_best + best[j] < best[i]:
                best[i] = c_best + best[j]
                choice[i] = (j, lp_best)
    out = []
    i = 0
    while i < NB:
        j, lp = choice[i]
        out.append((i, j - i, lp))
        i = j
    return out


def _pack(x, batch_idx, S, Wvals):
    """Sort+pack inputs. Returns (in_maps, sblocks, order)."""
    rps = SPB * NCORES                      # ranks per position
    NB = S // rps
    assert S % rps == 0, (S, rps)

    counts = np.bincount(batch_idx, minlength=S).astype(np.int64)
    order = np.argsort(-counts, kind="stable").astype(np.int64)
    sc = counts[order]
    Ls = np.maximum(sc[::rps], 1).astype(np.int64)        # [NB]
    sblocks = _partition(Ls)

    perm = np.argsort(batch_idx, kind="stable").astype(np.int64)
    seg_start = np.zeros(S + 1, np.int64)
    np.cumsum(counts, out=seg_start[1:])

    W0, W4 = float(Wvals[0]), float(Wvals[4])
    in_maps = [dict() for _ in range(NCORES)]
    W_total = int(sum(m * G * Lp for (_, m, Lp) in sblocks))
    xbf = x.astype(BF16)
    for c in range(NCORES):
        xcore = np.empty((128, W_total), BF16)
        aplane = np.empty((128, G * NB), np.float32)   # W0*n - W4*(Lp-n)*x0
        pplane = np.empty((128, G * NB), BF16)         # pad count Lp - n
        nplane = np.empty((128, G * NB), BF16)         # count n
        col = 0
        for (b0, m, Lp) in sblocks:
            Gm = m * G
            # ranks for positions b0..b0+m-1, concatenated: [m*SPB]
            ranks = (rps * (b0 + np.arange(m))[:, None]
                     + SPB * c + np.arange(SPB)[None, :]).ravel()
            segs = order[ranks]                            # [m*256]
            cnt = counts[segs]
            j = np.arange(Lp)[None, :]
            jeff = np.where(j < cnt[:, None], j, 0)        # replicate first row
            base = np.minimum(seg_start[segs], len(perm) - 1)  # empty-seg guard
            rows = perm[base[:, None] + jeff]              # [m*256, Lp]
            blk = xbf[rows]                                # [m*256, Lp, 64]
            # (b_rel, g, par, j, f) -> (par, f, j, b_rel, g)
            blk = blk.reshape(m, G, 2, Lp, F).transpose(2, 4, 3, 0, 1)
            xcore[:, col:col + Lp * Gm] = blk.reshape(128, Lp * Gm)
            cblk = cnt.reshape(m * G, 2).T                 # [2, m*G]
            sl = slice(b0 * G, b0 * G + Gm)
            # x0 in device layout [2, F, Gm]: value of the packed row j=0
            # (bf16-rounded, matching what the PE sum accumulates for pads)
            x0dev = np.asarray(
                blk[:, :, 0, :, :].reshape(128, Gm), np.float32)
            pads = np.broadcast_to(
                (float(Lp) - cblk)[:, None, :], (2, F, Gm)).reshape(128, Gm)
            aplane[:, sl] = (np.broadcast_to(
                (W0 * cblk)[:, None, :], (2, F, Gm)).reshape(128, Gm)
                - W4 * pads * x0dev)
            pplane[:, sl] = pads
            nplane[:, sl] = np.broadcast_to(
                cblk[:, None, :], (2, F, Gm)).reshape(128, Gm)
            col += Lp * Gm
        in_maps[c]["xb"] = xcore
        in_maps[c]["apl"] = aplane
        in_maps[c]["ppl"] = pplane
        in_maps[c]["npl"] = nplane
        # pre-scaled identities: every output term accumulates into one
        # PSUM tile per position via transpose-matmuls
        W1, W2, W3, W4 = [float(v) for v in Wvals[1:5]]
        eye = np.eye(128)
        in_maps[c]["iw1"] = (W1 * eye).astype(BF16)
        in_maps[c]["iw2"] = (W2 * eye).astype(BF16)
        in_maps[c]["iw3"] = (W3 * eye).astype(BF16)
        in_maps[c]["iw3n"] = (-W3 * eye).astype(BF16)
        in_maps[c]["iw4"] = (W4 * eye).astype(BF16)
        in_maps[c]["if32"] = eye.astype(np.float32)
    return in_maps, sblocks, order


def _tree(nc, pool, src_ap, L, Gm, dst_ap, op, bf16):
    """Pairwise-halving reduction tree over j (column-groups of Gm)."""
    assert L >= 2
    cur = src_ap
    Lc = L
    lvl = 0
    while Lc > 1:
        h = Lc // 2
        odd = Lc % 2 == 1
        if h == 1:
            nxt = dst_ap          # final level writes the stats plane
        else:
            t = pool.tile([128, h * Gm], bf16, tag=f"tr{lvl}")
            nxt = t[:]
        nc.vector.tensor_tensor(
            nxt[:, 0:h * Gm], cur[:, 0:h * Gm], cur[:, h * Gm:2 * h * Gm],
            op=op)
        if odd:
            nc.vector.tensor_tensor(
                nxt[:, 0:Gm], nxt[:, 0:Gm], cur[:, 2 * h * Gm:Lc * Gm], op=op)
        cur = nxt
        Lc = h
        lvl += 1


LAST_EXEC_NS = None
LAST_RESULTS = None


def kernel(x, batch_idx, max_index, t, W):
    global LAST_EXEC_NS, LAST_RESULTS
    x = np.ascontiguousarray(np.asarray(x, dtype=np.float32))
    bidx = np.asarray(batch_idx).astype(np.int64)
    S = int(max_index)
    t_np = np.asarray(t, dtype=np.float32).reshape(F)
    W_np = np.asarray(W, dtype=np.float32).reshape(-1)
    assert x.shape[1] == F and W_np.shape[0] == 5

    in_maps, sblocks, order = _pack(x, bidx, S, W_np)
    NB = S // (SPB * NCORES)
    tpar = np.tile(t_np, 2).reshape(128, 1).astype(np.float32)
    for m in in_maps:
        m["tpar"] = tpar

    nc = _build(sblocks, NB, W_np)

    if os.environ.get("KERNEL_SIM", "0") == "1":
        from concourse.bass_interp import CoreSim
        outs = []
        ncores = int(os.environ.get("KERNEL_SIM_CORES", str(NCORES)))
        for c in range(ncores):
            sim = CoreSim(nc, trace=False)
            for k, v in in_maps[c].items():
                sim.tensor(k)[:] = v
            sim.simulate(check_with_hw=False)
            outs.append({"ot": np.array(sim.tensor("ot"))})
        results = outs
        LAST_EXEC_NS = None
    else:
        from concourse import bass_utils
        trace = os.environ.get("KERNEL_TRACE", "0") == "1"
        tmpdir = os.environ.get("KERNEL_TRACE_DIR") or None
        res = bass_utils.run_bass_kernel_spmd(
            nc, in_maps, core_ids=list(range(NCORES)),
            trace=trace, tmpdir=tmpdir)
        results = res.results
        LAST_EXEC_NS = res.exec_time_ns
        LAST_RESULTS = res

    # Unpack: ot[g, b*128 + par*64 + f] -> [S, F] in original segment order
    # (rank i of position b = (g=i//2, par=i%2))
    rps = SPB * NCORES
    out_full = np.empty((S, F), np.float32)
    for c in range(len(results)):
        ot = np.asarray(results[c]["ot"])               # [128, NB*128]
        v = ot.reshape(128, NB, 2, F).transpose(1, 0, 2, 3)  # [NB, G, 2, F]
        v = v.reshape(NB * SPB, F)
        ranks = (rps * np.arange(NB)[:, None] + SPB * c
                 + np.arange(SPB)[None, :]).ravel()
        out_full[order[ranks]] = v

    # empty segments: reproduce the reference's identities exactly
    # (min=+inf, max=-inf, sums=relu_sum=n=0)
    counts = np.bincount(bidx, minlength=S)
    if counts.min() == 0:
        w = W_np.astype(np.float32)
        empty_val = (np.float32(w[1]) * np.float32(np.inf)
                     + np.float32(w[2]) * np.float32(-np.inf))
        out_full[counts == 0] = empty_val
    return out_full


def _build(sblocks, NB, Wvals):
    """Build the SPMD Bass graph. Returns compiled Bacc module."""
    import concourse.tile as tile
    from concourse import bacc, mybir

    f32 = mybir.dt.float32
    bf16 = mybir.dt.bfloat16
    OP = mybir.AluOpType

    SB = G * NB
    W_total = int(sum(m * G * Lp for (_, m, Lp) in sblocks))
    W0, W1, W2, W3, W4 = [float(v) for v in Wvals]

    nc = bacc.Bacc("TRN2", target_bir_lowering=False, debug=False,
                   num_devices=NCORES)
    xdr = nc.dram_tensor("xb", [128, W_total], bf16, kind="ExternalInput").ap()
    adr = nc.dram_tensor("apl", [128, SB], f32, kind="ExternalInput").ap()
    pdr = nc.dram_tensor("ppl", [128, SB], bf16, kind="ExternalInput").ap()
    ndr = nc.dram_tensor("npl", [128, SB], bf16, kind="ExternalInput").ap()
    tdr = nc.dram_tensor("tpar", [128, 1], f32, kind="ExternalInput").ap()
    iw = {k: nc.dram_tensor(k, [128, G],
                            f32 if k == "if32" else bf16,
                            kind="ExternalInput").ap()
          for k in ("iw1", "iw2", "iw3", "iw3n", "iw4", "if32")}
    otdr = nc.dram_tensor("ot", [128, NB * G], f32,
                          kind="ExternalOutput").ap()

    with tile.TileContext(nc) as tc, \
         tc.tile_pool(name="xpool", bufs=4) as xpool, \
         tc.tile_pool(name="tpool", bufs=1) as tpool, \
         tc.tile_pool(name="bpool", bufs=3) as bpool, \
         tc.tile_pool(name="pspool", bufs=8, space="PSUM") as pspool, \
         tc.tile_pool(name="cpool", bufs=1) as cpool:

        tpp = cpool.tile([128, 1], f32)
        nc.scalar.dma_start(tpp[:], tdr)
        tcl = cpool.tile([128, 1], f32)
        onemt = cpool.tile([128, 1], f32)

        iwt = {}
        for k, dr in iw.items():
            iwt[k] = cpool.tile([128, G], f32 if k == "if32" else bf16,
                                name=k + "_t", tag=k)
            nc.scalar.dma_start(iwt[k][:], dr)
        # plane DMAs are deferred to sbi==1 so they don't compete with the
        # cold-start x stream (they're first needed by relu_phase(0), which
        # runs two superblocks later)
        apl = cpool.tile([128, SB], f32)
        ppl = cpool.tile([128, SB], bf16)
        npl = cpool.tile([128, SB], bf16)

        # persistent stats planes (min/max are exact in bf16)
        mnall = cpool.tile([128, SB], bf16)
        mxall = cpool.tile([128, SB], bf16)

        col = 0
        pending = []   # (sl, b0, m, Lp, Gm, xt, bias, ptsb) awaiting phase 2

        def relu_phase(p):
            """In-place clamp, then PE relu-sums + corrections + combine."""
            sl, b0, m, Lp, Gm, xt, bias, ptsb = p
            # max trick, in place, in two j-halves so the PE chains below
            # can start on the first half early: xt <- max(xt, bias)
            hj = Lp // 2
            for (j0, j1) in ((0, hj), (hj, Lp)):
                if j0 == j1:
                    continue
                xjg = xt[:, j0 * Gm:j1 * Gm].rearrange(
                    "p (j g) -> p j g", g=Gm)
                bias_b = bias[:].unsqueeze(1).broadcast_to(
                    [128, j1 - j0, Gm])
                nc.vector.tensor_tensor(xjg, xjg, bias_b, op=OP.max)

            # correction planes: pads*max(x0,bias) and n*bias, folded into
            # the PE accumulation via the -W3 identity
            cr = bpool.tile([128, Gm], bf16, tag="cr")
            nc.vector.tensor_mul(cr[:], ppl[:, sl], xt[:, 0:Gm])
            nb = bpool.tile([128, Gm], bf16, tag="nb")
            nc.vector.tensor_mul(nb[:], npl[:, sl], bias[:])

            # PE: accumulate W3*(sum_j max(x,b) - cr - nb) + W1*mn + W2*mx
            # + apl into each position's PSUM region, then drain the
            # superblock's bank in one go
            for brel in range(m):
                pt = ptsb[brel][:]
                gsl = slice(brel * G, (brel + 1) * G)
                psl = slice(b0 * G + brel * G, b0 * G + (brel + 1) * G)
                for j in range(Lp):
                    xsl = xt[:, j * Gm + brel * G:j * Gm + brel * G + G]
                    nc.tensor.matmul(
                        out=pt, lhsT=xsl, rhs=iwt["iw3"][:],
                        start=False, stop=False)
                nc.tensor.matmul(
                    out=pt, lhsT=cr[:, gsl], rhs=iwt["iw3n"][:],
                    start=False, stop=False)
                nc.tensor.matmul(
                    out=pt, lhsT=nb[:, gsl], rhs=iwt["iw3n"][:],
                    start=False, stop=False)
                nc.tensor.matmul(
                    out=pt, lhsT=mnall[:, psl], rhs=iwt["iw1"][:],
                    start=False, stop=False)
                nc.tensor.matmul(
                    out=pt, lhsT=mxall[:, psl], rhs=iwt["iw2"][:],
                    start=False, stop=False)
                nc.tensor.matmul(
                    out=pt, lhsT=apl[:, psl], rhs=iwt["if32"][:],
                    start=False, stop=True)
                oto = bpool.tile([128, G], f32, tag="oto", name="oto")
                nc.scalar.copy(oto[:], ptsb[brel][:])
                pi = b0 + brel
                nc.sync.dma_start(otdr[:, pi * G:(pi + 1) * G], oto[:])

        for sbi, (b0, m, Lp) in enumerate(sblocks):
            Gm = m * G
            sl = slice(b0 * G, b0 * G + Gm)
            Wb = Lp * Gm
            xt = xpool.tile([128, Wb], bf16, tag="xt")
            first = sbi == 0 and Lp % 2 == 0
            if first:
                # split the first load in two so tree work can start after
                # the first half lands (hides half the cold-start DMA)
                hw_ = (Lp // 2) * Gm
                nc.sync.dma_start(xt[:, 0:hw_], xdr[:, col:col + hw_])
                nc.sync.dma_start(xt[:, hw_:Wb], xdr[:, col + hw_:col + Wb])
                for (dst, op, tg) in ((mnall, OP.min, "hn"),
                                      (mxall, OP.max, "hx")):
                    ha = bpool.tile([128, Gm], bf16, tag=tg + "a")
                    hb = bpool.tile([128, Gm], bf16, tag=tg + "b")
                    _tree(nc, tpool, xt[:, 0:hw_], Lp // 2, Gm, ha[:], op, bf16)
                    _tree(nc, tpool, xt[:, hw_:Wb], Lp // 2, Gm, hb[:], op, bf16)
                    nc.vector.tensor_tensor(dst[:, sl], ha[:], hb[:], op=op)
            else:
                deng = nc.sync
                deng.dma_start(xt[:], xdr[:, col:col + Wb])
                _tree(nc, tpool, xt[:], Lp, Gm, mnall[:, sl], OP.min, bf16)
                _tree(nc, tpool, xt[:], Lp, Gm, mxall[:, sl], OP.max, bf16)
            col += Wb

            # PE raw sums: accumulate W4 * transposes of each j-slice per
            # position: pt[g, (par, f)] += W4 * x[(par, f), (j, brel, g)]
            # (one PSUM bank per position)
            ptsb = []
            for brel in range(m):
                pt = pspool.tile([128, G], f32, tag="pt", name="pt")
                ptsb.append(pt)
                for j in range(Lp):
                    xsl = xt[:, j * Gm + brel * G:j * Gm + brel * G + G]
                    nc.tensor.matmul(
                        out=pt[:], lhsT=xsl, rhs=iwt["iw4"][:],
                        start=(j == 0), stop=False)

            if sbi == 0:
                # t-clamp ops deferred here: the DVE stream is in-order, so
                # emitting them first would stall DVE on the tpar DMA
                nc.vector.tensor_scalar(tcl[:], tpp[:], 0.0, 1.0,
                                        OP.max, OP.min)
                nc.vector.tensor_scalar(onemt[:], tcl[:], -1.0, 1.0,
                                        OP.mult, OP.add)
            elif sbi == 1:
                nc.scalar.dma_start(ppl[:], pdr)
                nc.scalar.dma_start(npl[:], ndr)
                nc.scalar.dma_start(apl[:], adr)

            # bias = t*mx + (1-t)*mn  (bf16)
            biasA = bpool.tile([128, Gm], bf16, tag="biasA")
            nc.vector.tensor_scalar_mul(biasA[:], mxall[:, sl], tcl[:])
            bias = bpool.tile([128, Gm], bf16, tag="bias")
            nc.vector.scalar_tensor_tensor(
                bias[:], mnall[:, sl], onemt[:], biasA[:], OP.mult, OP.add)

            # defer this superblock's clamp+relu two iterations so the PE
            # transpose-sum chains (reading xt) have a two-tree window to
            # drain before the in-place clamp needs the tile
            pending.append((sl, b0, m, Lp, Gm, xt, bias, ptsb))
            if len(pending) > 2:
                relu_phase(pending.pop(0))

        for p in pending:
            relu_phase(p)

    nc.compile()
    return nc


# revision 46
# speedup vs baseline: 1.1595x; 1.1595x over previous
"""Trainium2 Bass kernel for AdaptiveReLU segment-reduce.

Reference computation (per segment s over instance rows x[i] with batch_idx[i]==s):
    mn = min, mx = max, sums = sum, n = count
    bias = t*mx + (1-t)*mn            (t clamped to [0,1], per feature)
    relu_sum = sum(relu(x - bias))
    out[s,f] = W0*n + W1*mn + W2*mx + W3*relu_sum + W4*sums

Strategy: host-side sort + count-sorted packing so every segment lives on one
core with a few % padding, then a fully local (collective-free) SPMD kernel on
8 NeuronCores.

Work split across engines:
  - DVE (feature layout, [128=(parity,feat), (j, block, g)] bf16): pairwise
    min/max trees, in-place clamp max(x, bias) (broadcast runs at 2x), and
    the relu-sum tree.  The clamp+relu phase of superblock k is deferred
    until after superblock k+1's trees so the PE path below can finish
    reading x tiles before they are clamped in place.
  - PE: raw segment sums via transpose-accumulate matmuls: for each
    position, sum_j transpose(x[:, j, :]) with an identity moving operand
    accumulates [seg-group, (parity, feat)] sums in PSUM, fp32-exact.
    Pad rows (which replicate row 0) are corrected via the host apl plane.
  - ACT: PSUM drains + secondary DMA queue.
  - Host: pack/unpack and the final out += W4*raw_sums add during unpack.
"""

import os
import numpy as np
import ml_dtypes

F = 64            # feature dim
G = 128           # segment-groups per position (2 parities x 64 features)
SPB = 2 * G       # segments per position per core
NCORES = 8
MAX_LM = 80       # SBUF cap: L * m <= MAX_LM  (tile = L*m*128 cols bf16)

BF16 = ml_dtypes.bfloat16


def _nfolds(L):
    n = 0
    while L > 1:
        if L % 2:
            n += 1
        L //= 2
    return n


def _partition(Ls):
    """DP partition of block positions into superblocks.

    Returns list of (start, m, Lpad).  Cost model (ns):
      padding: 270 per extra L-unit per position
      folds:   3 trees * (m*64 + 220) per odd level
      fixed:   5000 per superblock
    """
    NB = len(Ls)
    INF = float("inf")
    best = [INF] * (NB + 1)
    choice = [None] * (NB + 1)
    best[NB] = 0.0
    for i in range(NB - 1, -1, -1):
        for j in range(i + 1, NB + 1):
            m = j - i
            Lmax = int(Ls[i])
            if Lmax * m > MAX_LM:
                break
            c_best = INF
            lp_best = Lmax
            for Lp in range(Lmax, min(Lmax + 13, MAX_LM // m + 1)):
                pad = sum(Lp - int(Ls[k]) for k in range(i, j))
                c = pad * 270.0 + _nfolds(Lp) * 3 * (m * 64 + 220) + 5000.0
                if c < c_best:
                    c_best, lp_best = c, Lp
            if c_best + best[j] < best[i]:
                best[i] = c_best + best[j]
                choice[i] = (j, lp_best)
    out = []
    i = 0
    while i < NB:
        j, lp = choice[i]
        out.append((i, j - i, lp))
        i = j
    return out


def _pack(x, batch_idx, S, Wvals):
    """Sort+pack inputs. Returns (in_maps, sblocks, order)."""
    rps = SPB * NCORES                      # ranks per position
    NB = S // rps
    assert S % rps == 0, (S, rps)

    counts = np.bincount(batch_idx, minlength=S).astype(np.int64)
    order = np.argsort(-counts, kind="stable").astype(np.int64)
    sc = counts[order]
    Ls = np.maximum(sc[::rps], 1).astype(np.int64)        # [NB]
    sblocks = _partition(Ls)

    perm = np.argsort(batch_idx, kind="stable").astype(np.int64)
    seg_start = np.zeros(S + 1, np.int64)
    np.cumsum(counts, out=seg_start[1:])

    W0, W4 = float(Wvals[0]), float(Wvals[4])
    in_maps = [dict() for _ in range(NCORES)]
    W_total = int(sum(m * G * Lp for (_, m, Lp) in sblocks))
    xbf = x.astype(BF16)
    for c in range(NCORES):
        xcore = np.empty((128, W_total), BF16)
        aplane = np.empty((128, G * NB), np.float32)   # W0*n - W4*(Lp-n)*x0
        pplane = np.empty((128, G * NB), BF16)         # pad count Lp - n
        nplane = np.empty((128, G * NB), BF16)         # count n
        col = 0
        for (b0, m, Lp) in sblocks:
            Gm = m * G
            # ranks for positions b0..b0+m-1, concatenated: [m*SPB]
            ranks = (rps * (b0 + np.arange(m))[:, None]
                     + SPB * c + np.arange(SPB)[None, :]).ravel()
            segs = order[ranks]                            # [m*256]
            cnt = counts[segs]
            j = np.arange(Lp)[None, :]
            jeff = np.where(j < cnt[:, None], j, 0)        # replicate first row
            base = np.minimum(seg_start[segs], len(perm) - 1)  # empty-seg guard
            rows = perm[base[:, None] + jeff]              # [m*256, Lp]
            blk = xbf[rows]                                # [m*256, Lp, 64]
            # (b_rel, g, par, j, f) -> (par, f, j, b_rel, g)
            blk = blk.reshape(m, G, 2, Lp, F).transpose(2, 4, 3, 0, 1)
            xcore[:, col:col + Lp * Gm] = blk.reshape(128, Lp * Gm)
            cblk = cnt.reshape(m * G, 2).T                 # [2, m*G]
            sl = slice(b0 * G, b0 * G + Gm)
            # x0 in device layout [2, F, Gm]: value of the packed row j=0
            # (bf16-rounded, matching what the PE sum accumulates for pads)
            x0dev = np.asarray(
                blk[:, :, 0, :, :].reshape(128, Gm), np.float32)
            pads = np.broadcast_to(
                (float(Lp) - cblk)[:, None, :], (2, F, Gm)).reshape(128, Gm)
            aplane[:, sl] = (np.broadcast_to(
                (W0 * cblk)[:, None, :], (2, F, Gm)).reshape(128, Gm)
                - W4 * pads * x0dev)
            pplane[:, sl] = pads
            nplane[:, sl] = np.broadcast_to(
                cblk[:, None, :], (2, F, Gm)).reshape(128, Gm)
            col += Lp * Gm
        in_maps[c]["xb"] = xcore
        in_maps[c]["apl"] = aplane
        in_maps[c]["ppl"] = pplane
        in_maps[c]["npl"] = nplane
        # pre-scaled identities: every output term accumulates into one
        # PSUM tile per position via transpose-matmuls
        W1, W2, W3, W4 = [float(v) for v in Wvals[1:5]]
        eye = np.eye(128)
        in_maps[c]["iw1"] = (W1 * eye).astype(BF16)
        in_maps[c]["iw2"] = (W2 * eye).astype(BF16)
        in_maps[c]["iw3"] = (W3 * eye).astype(BF16)
        in_maps[c]["iw3n"] = (-W3 * eye).astype(BF16)
        in_maps[c]["iw4"] = (W4 * eye).astype(BF16)
        in_maps[c]["if32"] = eye.astype(np.float32)
    return in_maps, sblocks, order


def _tree(nc, pool, src_ap, L, Gm, dst_ap, op, bf16):
    """Pairwise-halving reduction tree over j (column-groups of Gm)."""
    assert L >= 2
    cur = src_ap
    Lc = L
    lvl = 0
    while Lc > 1:
        h = Lc // 2
        odd = Lc % 2 == 1
        if h == 1:
            nxt = dst_ap          # final level writes the stats plane
        else:
            t = pool.tile([128, h * Gm], bf16, tag=f"tr{lvl}")
            nxt = t[:]
        nc.vector.tensor_tensor(
            nxt[:, 0:h * Gm], cur[:, 0:h * Gm], cur[:, h * Gm:2 * h * Gm],
            op=op)
        if odd:
            nc.vector.tensor_tensor(
                nxt[:, 0:Gm], nxt[:, 0:Gm], cur[:, 2 * h * Gm:Lc * Gm], op=op)
        cur = nxt
        Lc = h
        lvl += 1


LAST_EXEC_NS = None
LAST_RESULTS = None


def kernel(x, batch_idx, max_index, t, W):
    global LAST_EXEC_NS, LAST_RESULTS
    x = np.ascontiguousarray(np.asarray(x, dtype=np.float32))
    bidx = np.asarray(batch_idx).astype(np.int64)
    S = int(max_index)
    t_np = np.asarray(t, dtype=np.float32).reshape(F)
    W_np = np.asarray(W, dtype=np.float32).reshape(-1)
    assert x.shape[1] == F and W_np.shape[0] == 5

    in_maps, sblocks, order = _pack(x, bidx, S, W_np)
    NB = S // (SPB * NCORES)
    tpar = np.tile(t_np, 2).reshape(128, 1).astype(np.float32)
    for m in in_maps:
        m["tpar"] = tpar

    nc = _build(sblocks, NB, W_np)

    if os.environ.get("KERNEL_SIM", "0") == "1":
        from concourse.bass_interp import CoreSim
        outs = []
        ncores = int(os.environ.get("KERNEL_SIM_CORES", str(NCORES)))
        for c in range(ncores):
            sim = CoreSim(nc, trace=False)
            for k, v in in_maps[c].items():
                sim.tensor(k)[:] = v
            sim.simulate(check_with_hw=False)
            outs.append({"ot": np.array(sim.tensor("ot"))})
        results = outs
        LAST_EXEC_NS = None
    else:
        from concourse import bass_utils
        trace = os.environ.get("KERNEL_TRACE", "0") == "1"
        tmpdir = os.environ.get("KERNEL_TRACE_DIR") or None
        res = bass_utils.run_bass_kernel_spmd(
            nc, in_maps, core_ids=list(range(NCORES)),
            trace=trace, tmpdir=tmpdir)
        results = res.results
        LAST_EXEC_NS = res.exec_time_ns
        LAST_RESULTS = res

    # Unpack: ot[g, b*128 + par*64 + f] -> [S, F] in original segment order
    # (rank i of position b = (g=i//2, par=i%2))
    rps = SPB * NCORES
    out_full = np.empty((S, F), np.float32)
    for c in range(len(results)):
        ot = np.asarray(results[c]["ot"])               # [128, NB*128]
        v = ot.reshape(128, NB, 2, F).transpose(1, 0, 2, 3)  # [NB, G, 2, F]
        v = v.reshape(NB * SPB, F)
        ranks = (rps * np.arange(NB)[:, None] + SPB * c
                 + np.arange(SPB)[None, :]).ravel()
        out_full[order[ranks]] = v

    # empty segments: reproduce the reference's identities exactly
    # (min=+inf, max=-inf, sums=relu_sum=n=0)
    counts = np.bincount(bidx, minlength=S)
    if counts.min() == 0:
        w = W_np.astype(np.float32)
        empty_val = (np.float32(w[1]) * np.float32(np.inf)
                     + np.float32(w[2]) * np.float32(-np.inf))
        out_full[counts == 0] = empty_val
    return out_full


def _build(sblocks, NB, Wvals):
    """Build the SPMD Bass graph. Returns compiled Bacc module."""
    import concourse.tile as tile
    from concourse import bacc, mybir

    f32 = mybir.dt.float32
    bf16 = mybir.dt.bfloat16
    OP = mybir.AluOpType

    SB = G * NB
    W_total = int(sum(m * G * Lp for (_, m, Lp) in sblocks))
    W0, W1, W2, W3, W4 = [float(v) for v in Wvals]

    nc = bacc.Bacc("TRN2", target_bir_lowering=False, debug=False,
                   num_devices=NCORES)
    xdr = nc.dram_tensor("xb", [128, W_total], bf16, kind="ExternalInput").ap()
    adr = nc.dram_tensor("apl", [128, SB], f32, kind="ExternalInput").ap()
    pdr = nc.dram_tensor("ppl", [128, SB], bf16, kind="ExternalInput").ap()
    ndr = nc.dram_tensor("npl", [128, SB], bf16, kind="ExternalInput").ap()
    tdr = nc.dram_tensor("tpar", [128, 1], f32, kind="ExternalInput").ap()
    iw = {k: nc.dram_tensor(k, [128, G],
                            f32 if k == "if32" else bf16,
                            kind="ExternalInput").ap()
          for k in ("iw1", "iw2", "iw3", "iw3n", "iw4", "if32")}
    otdr = nc.dram_tensor("ot", [128, NB * G], f32,
                          kind="ExternalOutput").ap()

    with tile.TileContext(nc) as tc, \
         tc.tile_pool(name="xpool", bufs=4) as xpool, \
         tc.tile_pool(name="tpool", bufs=1) as tpool, \
         tc.tile_pool(name="bpool", bufs=3) as bpool, \
         tc.tile_pool(name="pspool", bufs=8, space="PSUM") as pspool, \
         tc.tile_pool(name="cpool", bufs=1) as cpool:

        tpp = cpool.tile([128, 1], f32)
        nc.scalar.dma_start(tpp[:], tdr)
        tcl = cpool.tile([128, 1], f32)
        onemt = cpool.tile([128, 1], f32)

        iwt = {}
        for k, dr in iw.items():
            iwt[k] = cpool.tile([128, G], f32 if k == "if32" else bf16,
                                name=k + "_t", tag=k)
            nc.scalar.dma_start(iwt[k][:], dr)
        # plane DMAs are deferred to sbi==1 so they don't compete with the
        # cold-start x stream (they're first needed by relu_phase(0), which
        # runs two superblocks later)
        apl = cpool.tile([128, SB], f32)
        ppl = cpool.tile([128, SB], bf16)
        npl = cpool.tile([128, SB], bf16)

        # persistent stats planes (min/max are exact in bf16)
        mnall = cpool.tile([128, SB], bf16)
        mxall = cpool.tile([128, SB], bf16)

        col = 0
        pending = []   # (sl, b0, m, Lp, Gm, xt, bias, ptsb) awaiting phase 2

        def relu_phase(p):
            """In-place clamp, then PE relu-sums + corrections + combine."""
            sl, b0, m, Lp, Gm, xt, bias, ptsb = p
            # max trick, in place: xt <- max(xt, bias)
            xjg = xt[:].rearrange("p (j g) -> p j g", g=Gm)
            bias_b = bias[:].unsqueeze(1).broadcast_to([128, Lp, Gm])
            nc.vector.tensor_tensor(xjg, xjg, bias_b, op=OP.max)

            # correction planes: pads*max(x0,bias) and n*bias, folded into
            # the PE accumulation via the -W3 identity
            cr = bpool.tile([128, Gm], bf16, tag="cr")
            nc.vector.tensor_mul(cr[:], ppl[:, sl], xt[:, 0:Gm])
            nb = bpool.tile([128, Gm], bf16, tag="nb")
            nc.vector.tensor_mul(nb[:], npl[:, sl], bias[:])

            # PE: accumulate W3*(sum_j max(x,b) - cr - nb) + W1*mn + W2*mx
            # + apl into each position's PSUM region, then drain the
            # superblock's bank in one go
            for brel in range(m):
                pt = ptsb[brel][:]
                gsl = slice(brel * G, (brel + 1) * G)
                psl = slice(b0 * G + brel * G, b0 * G + (brel + 1) * G)
                for j in range(Lp):
                    xsl = xt[:, j * Gm + brel * G:j * Gm + brel * G + G]
                    nc.tensor.matmul(
                        out=pt, lhsT=xsl, rhs=iwt["iw3"][:],
                        start=False, stop=False)
                nc.tensor.matmul(
                    out=pt, lhsT=cr[:, gsl], rhs=iwt["iw3n"][:],
                    start=False, stop=False)
                nc.tensor.matmul(
                    out=pt, lhsT=nb[:, gsl], rhs=iwt["iw3n"][:],
                    start=False, stop=False)
                nc.tensor.matmul(
                    out=pt, lhsT=mnall[:, psl], rhs=iwt["iw1"][:],
                    start=False, stop=False)
                nc.tensor.matmul(
                    out=pt, lhsT=mxall[:, psl], rhs=iwt["iw2"][:],
                    start=False, stop=False)
                nc.tensor.matmul(
                    out=pt, lhsT=apl[:, psl], rhs=iwt["if32"][:],
                    start=False, stop=True)
                oto = bpool.tile([128, G], f32, tag="oto", name="oto")
                nc.scalar.copy(oto[:], ptsb[brel][:])
                pi = b0 + brel
                nc.sync.dma_start(otdr[:, pi * G:(pi + 1) * G], oto[:])

        for sbi, (b0, m, Lp) in enumerate(sblocks):
            Gm = m * G
            sl = slice(b0 * G, b0 * G + Gm)
            Wb = Lp * Gm
            xt = xpool.tile([128, Wb], bf16, tag="xt")
            first = sbi == 0 and Lp % 2 == 0
            if first:
                # split the first load in two so tree work can start after
                # the first half lands (hides half the cold-start DMA)
                hw_ = (Lp // 2) * Gm
                nc.sync.dma_start(xt[:, 0:hw_], xdr[:, col:col + hw_])
                nc.sync.dma_start(xt[:, hw_:Wb], xdr[:, col + hw_:col + Wb])
                for (dst, op, tg) in ((mnall, OP.min, "hn"),
                                      (mxall, OP.max, "hx")):
                    ha = bpool.tile([128, Gm], bf16, tag=tg + "a")
                    hb = bpool.tile([128, Gm], bf16, tag=tg + "b")
                    _tree(nc, tpool, xt[:, 0:hw_], Lp // 2, Gm, ha[:], op, bf16)
                    _tree(nc, tpool, xt[:, hw_:Wb], Lp // 2, Gm, hb[:], op, bf16)
                    nc.vector.tensor_tensor(dst[:, sl], ha[:], hb[:], op=op)
            else:
                deng = nc.sync
                deng.dma_start(xt[:], xdr[:, col:col + Wb])
                _tree(nc, tpool, xt[:], Lp, Gm, mnall[:, sl], OP.min, bf16)
                _tree(nc, tpool, xt[:], Lp, Gm, mxall[:, sl], OP.max, bf16)
            col += Wb

            # PE raw sums: accumulate W4 * transposes of each j-slice per
            # position: pt[g, (par, f)] += W4 * x[(par, f), (j, brel, g)]
            # (one PSUM bank per position)
            ptsb = []
            for brel in range(m):
                pt = pspool.tile([128, G], f32, tag="pt", name="pt")
                ptsb.append(pt)
                for j in range(Lp):
                    xsl = xt[:, j * Gm + brel * G:j * Gm + brel * G + G]
                    nc.tensor.matmul(
                        out=pt[:], lhsT=xsl, rhs=iwt["iw4"][:],
                        start=(j == 0), stop=False)

            if sbi == 0:
                # t-clamp ops deferred here: the DVE stream is in-order, so
                # emitting them first would stall DVE on the tpar DMA
                nc.vector.tensor_scalar(tcl[:], tpp[:], 0.0, 1.0,
                                        OP.max, OP.min)
                nc.vector.tensor_scalar(onemt[:], tcl[:], -1.0, 1.0,
                                        OP.mult, OP.add)
            elif sbi == 1:
                nc.scalar.dma_start(ppl[:], pdr)
                nc.scalar.dma_start(npl[:], ndr)
                nc.scalar.dma_start(apl[:], adr)

            # bias = t*mx + (1-t)*mn  (bf16)
            biasA = bpool.tile([128, Gm], bf16, tag="biasA")
            nc.vector.tensor_scalar_mul(biasA[:], mxall[:, sl], tcl[:])
            bias = bpool.tile([128, Gm], bf16, tag="bias")
            nc.vector.scalar_tensor_tensor(
                bias[:], mnall[:, sl], onemt[:], biasA[:], OP.mult, OP.add)

            # defer this superblock's clamp+relu two iterations so the PE
            # transpose-sum chains (reading xt) have a two-tree window to
            # drain before the in-place clamp needs the tile
            pending.append((sl, b0, m, Lp, Gm, xt, bias, ptsb))
            if len(pending) > 2:
                relu_phase(pending.pop(0))

        for p in pending:
            relu_phase(p)

    nc.compile()
    return nc
